# revision 13
# baseline (speedup 1.0000x reference)
"""Trainium2 Bass kernel for nn_MatchingMetric (epipolar matching metric).

Strategy: shard the query-pixel axis (first H*W) across 8 NeuronCores; each
core computes 288 query rows of both (HW, HW) softmax tensors plus the top-8
candidate argmax indices per row.  Per chunk of <=128 query rows:
    z[q,p]  = -d(q,p)^2                      (K=60 quad-split bf16 matmul)
    S[q,p]  = <f_q, g_p>                     (fp16 matmul, fp32 accumulate)
    dwc     = exp(z) * S                     (ACT exp + DVE multiply)
    probs   = exp(dwc) / sum_p exp(dwc)      (ACT with fused row-sum,
                                              GPSIMD normalize_recip)
    top-8 of dwc (values+indices)            (DVE max8 / max_index)
The tiny camera math, feature normalization, exact rescoring of the 8
argmax candidates per row, and the index->grid lookup run on the host.
"""
import sys

sys.path.insert(0, "/opt/trn_rl_repo")

import numpy as np
import ml_dtypes
from contextlib import ExitStack

import concourse.bacc as bacc
import concourse.tile as tile
import concourse.mybir as mybir
from concourse.bass_utils import run_bass_kernel_spmd

B, C, H, W = 1, 256, 48, 48
HW = H * W
EPS = 1e-6
NCORES = 8
QPC = HW // NCORES            # 288 query rows per core
NSLC = [(0, 512), (512, 512), (1024, 512), (1536, 512), (2048, 256)]
ZK = 60                       # K of the z matmul (10 split pairs x 6 monomials)

LAST_EXEC_NS = None
_CACHED = {}


# --------------------------------------------------------------------------
# host-side math
# --------------------------------------------------------------------------
def _bf16(x):
    return np.asarray(x, ml_dtypes.bfloat16)


def _epiline_geometry(K1, K2, c2w1, c2w2):
    """Folded-normalized epiline coefficients per query pixel (float64)."""
    def proj(K, c2w):
        R = c2w[:3, :3]
        t = c2w[:3, 3:4]
        w2c = np.concatenate([R.T, -R.T @ t], axis=1)
        return K @ w2c

    def fundamental(P1, P2):
        rows = [np.array([1, 2]), np.array([2, 0]), np.array([0, 1])]
        X = [P1[r, :] for r in rows]
        Y = [P2[r, :] for r in rows]
        mats = np.stack(
            [np.concatenate([X[j], Y[i]], 0) for i in range(3) for j in range(3)], 0
        )
        return np.linalg.det(mats).reshape(3, 3)

    P1 = proj(K1.astype(np.float64), c2w1.astype(np.float64))
    P2 = proj(K2.astype(np.float64), c2w2.astype(np.float64))
    F = fundamental(P1, P2)

    xs, ys = np.meshgrid(np.arange(W), np.arange(H))
    pts = np.stack([xs, ys], -1).reshape(-1, 2).astype(np.float64) + 0.5
    ph = np.concatenate([pts, np.ones((HW, 1))], 1)

    def lines_of(Fm):
        ln = ph @ Fm.T
        ln = ln / np.sqrt(ln[:, 0] ** 2 + ln[:, 1] ** 2)[:, None]
        ln = ln / np.sqrt(ln[:, 0] ** 2 + ln[:, 1] ** 2 + 1e-9)[:, None]
        return ln

    L21 = lines_of(F)       # lines indexed by q for direction 2->1
    L12 = lines_of(F.T)     # direction 1->2
    return L12, L21, pts


def _quad_coeffs(lines):
    a, b, c = lines[:, 0], lines[:, 1], lines[:, 2]
    x0, y0 = W / 2.0, H / 2.0
    ct = c + a * x0 + b * y0
    return np.stack([-a * a, -b * b, -ct * ct, -2 * a * b, -2 * a * ct, -2 * b * ct], 0)


def _monomials(pts):
    x0, y0 = W / 2.0, H / 2.0
    xt = pts[:, 0] - x0
    yt = pts[:, 1] - y0
    return np.stack([xt * xt, yt * yt, np.ones(HW), xt * yt, xt, yt], 0)


_SPLIT_PAIRS = [(i, j) for i in range(4) for j in range(4) if i + j < 4]  # 10 pairs


def _quad_split_stack(x64, pick):
    """4-level bf16 split of x64 (6, N); stack the `pick` level per pair -> (60, N)."""
    levels = []
    r = np.array(x64, np.float64)
    for _ in range(4):
        h = _bf16(r).astype(np.float64)
        levels.append(_bf16(r))
        r = r - h
    return np.concatenate([levels[p[pick]] for p in _SPLIT_PAIRS], 0)


def _normalized_features(features1, features2):
    f1 = np.asarray(features1, np.float32)[0].reshape(C, HW) + np.float32(EPS)
    f2 = np.asarray(features2, np.float32)[0].reshape(C, HW) + np.float32(EPS)
    n1 = np.sqrt((f1 ** 2).sum(0, dtype=np.float32))
    n2 = np.sqrt((f2 ** 2).sum(0, dtype=np.float32))
    return f1 / n1[None, :], f2 / n2[None, :]


# --------------------------------------------------------------------------
# device program (SPMD, one Bacc module shared by all 8 cores)
# --------------------------------------------------------------------------
def _build_module():
    nc = bacc.Bacc("TRN2", num_devices=NCORES)
    f16 = mybir.dt.float16
    bf16 = mybir.dt.bfloat16
    f32 = mybir.dt.float32
    u32 = mybir.dt.uint32

    ins = {}
    for nm, shape, dt in [
        ("h1", [C, HW], f16), ("h2", [C, HW], f16),
        ("h1q", [C, QPC], f16), ("h2q", [C, QPC], f16),
        ("uz", [ZK, HW], bf16),
        ("wq12", [ZK, QPC], bf16), ("wq21", [ZK, QPC], bf16),
    ]:
        ins[nm] = nc.declare_dram_parameter(nm, shape, dt, isOutput=False)

    outs = {}
    for nm, shape, dt in [
        ("p12", [QPC, HW], f32), ("p21", [QPC, HW], f32),
        ("i12", [QPC, 8], u32), ("i21", [QPC, 8], u32),
    ]:
        outs[nm] = nc.declare_dram_parameter(nm, shape, dt, isOutput=True)

    # chunk = list of (direction, q-start, partition-base, rows)
    CHUNKS = [
        [(0, 0, 0, 128)],
        [(0, 128, 0, 128)],
        [(1, 0, 0, 128)],
        [(1, 128, 0, 128)],
        [(0, 256, 0, 32), (1, 256, 32, 32)],   # merged tails
    ]
    DIR = [("wq12", "h1q", "h2", "p12", "i12"), ("wq21", "h2q", "h1", "p21", "i21")]

    with tile.TileContext(nc) as tc, ExitStack() as ctx:
        cpool = ctx.enter_context(tc.tile_pool(name="consts", bufs=1))
        wpool = ctx.enter_context(tc.tile_pool(name="work", bufs=3))
        spool = ctx.enter_context(tc.tile_pool(name="small", bufs=4))
        psz = ctx.enter_context(tc.tile_pool(name="psz", bufs=1, space="PSUM"))
        pss = ctx.enter_context(tc.tile_pool(name="pss", bufs=3, space="PSUM"))

        # loads in first-use order: chunk 0 (dir 1->2) needs wq12, uz, h1q, h2
        sb = {}

        def load_wq(nm):
            t = cpool.tile([ZK, QPC], bf16, name=nm)
            nc.sync.dma_start(t[:], ins[nm][:, :])
            sb[nm] = t

        def load_feat(nm, width):
            for k in range(2):
                t = cpool.tile([128, width], f16, name=f"{nm}_{k}")
                nc.sync.dma_start(t[:], ins[nm][k * 128:(k + 1) * 128, :])
                sb[f"{nm}_{k}"] = t

        load_wq("wq12")
        uz = cpool.tile([ZK, HW], bf16, name="uz")
        nc.sync.dma_start(uz[:], ins["uz"][:, :])
        load_feat("h1q", QPC)
        # h2 is on chunk 0's critical path: land its first half early
        for k in range(2):
            t = cpool.tile([128, HW], f16, name=f"h2_{k}")
            nc.sync.dma_start(t[:, :1024], ins["h2"][k * 128:(k + 1) * 128, :1024])
            nc.sync.dma_start(t[:, 1024:], ins["h2"][k * 128:(k + 1) * 128, 1024:])
            sb[f"h2_{k}"] = t
        load_wq("wq21")
        load_feat("h2q", QPC)
        load_feat("h1", HW)

        for ci, parts in enumerate(CHUNKS):
            sfx = f"c{ci}"
            P = sum(pp[3] for pp in parts)       # total partitions used

            # --- z matmul -> PSUM ---
            zp = psz.tile([128, HW], f32, tag="z", name=f"z_{sfx}")
            for d, qs, pb, rows in parts:
                wqd = sb[DIR[d][0]]
                for ns, nw in NSLC:
                    nc.tensor.matmul(
                        zp[pb:pb + rows, ns:ns + nw],
                        wqd[:, qs:qs + rows],
                        uz[:, ns:ns + nw],
                        start=True, stop=True,
                    )
            # --- e = exp(z) ---
            e = wpool.tile([128, HW], f32, tag="e", name=f"e_{sfx}")
            nc.scalar.activation(e[:P, :], zp[:P, :], mybir.ActivationFunctionType.Exp)

            # --- S matmul (fp16) + dwc = e * S per n-slice ---
            dwc = wpool.tile([128, HW], f32, tag="dwc", name=f"dwc_{sfx}")
            for ns, nw in NSLC:
                sp = pss.tile([128, 512], f32, tag="s", name=f"s_{sfx}_{ns}")
                for d, qs, pb, rows in parts:
                    _, lhs, rhs = DIR[d][0], DIR[d][1], DIR[d][2]
                    for k in range(2):
                        nc.tensor.matmul(
                            sp[pb:pb + rows, :nw],
                            sb[f"{lhs}_{k}"][:, qs:qs + rows],
                            sb[f"{rhs}_{k}"][:, ns:ns + nw],
                            start=(k == 0), stop=(k == 1),
                        )
                nc.vector.tensor_tensor(
                    dwc[:P, ns:ns + nw], e[:P, ns:ns + nw], sp[:P, :nw],
                    op=mybir.AluOpType.mult,
                )

            # --- top-8 of dwc (overlaps the exp/normalize chain) ---
            mx = spool.tile([128, 8], f32, tag="mx", name=f"mx_{sfx}")
            mi = spool.tile([128, 8], u32, tag="mi", name=f"mi_{sfx}")
            nc.vector.max_with_indices(mx[:P, :], mi[:P, :], dwc[:P, :])

            # --- u = exp(dwc), s = row sum (fused) ---
            u = wpool.tile([128, HW], f32, tag="u", name=f"u_{sfx}")
            s = spool.tile([128, 1], f32, tag="sum", name=f"sum_{sfx}")
            nc.scalar.activation(
                u[:P, :], dwc[:P, :], mybir.ActivationFunctionType.Exp,
                accum_out=s[:P, :],
            )

            # --- probs = u / s (gpsimd) ---
            p = wpool.tile([128, HW], f32, tag="p", name=f"p_{sfx}")
            nc.gpsimd.normalize_recip(p[:P, :], u[:P, :], s[:P, :])

            for d, qs, pb, rows in parts:
                nc.sync.dma_start(outs[DIR[d][3]][qs:qs + rows, :], p[pb:pb + rows, :])
                nc.sync.dma_start(outs[DIR[d][4]][qs:qs + rows, :], mi[pb:pb + rows, :])

    nc.finalize()
    return nc


def _get_module():
    if "nc" not in _CACHED:
        _CACHED["nc"] = _build_module()
    return _CACHED["nc"]


# --------------------------------------------------------------------------
# entry point
# --------------------------------------------------------------------------
def kernel(features1, features2, K1, K2, c2w1, c2w2):
    global LAST_EXEC_NS
    K1 = np.asarray(K1, np.float32)[0]
    K2 = np.asarray(K2, np.float32)[0]
    c2w1 = np.asarray(c2w1, np.float32)[0]
    c2w2 = np.asarray(c2w2, np.float32)[0]

    L12, L21, pts = _epiline_geometry(K1, K2, c2w1, c2w2)
    w12 = _quad_coeffs(L12)
    w21 = _quad_coeffs(L21)
    umon = _monomials(pts)
    wz12 = _quad_split_stack(w12, 0)
    wz21 = _quad_split_stack(w21, 0)
    uzs = _quad_split_stack(umon, 1)

    f1n, f2n = _normalized_features(features1, features2)
    h1 = f1n.astype(np.float16)
    h2 = f2n.astype(np.float16)

    in_maps = []
    for cid in range(NCORES):
        sl = slice(cid * QPC, (cid + 1) * QPC)
        in_maps.append({
            "h1": h1, "h2": h2,
            "h1q": np.ascontiguousarray(h1[:, sl]),
            "h2q": np.ascontiguousarray(h2[:, sl]),
            "uz": uzs,
            "wq12": np.ascontiguousarray(wz12[:, sl]),
            "wq21": np.ascontiguousarray(wz21[:, sl]),
        })

    nc = _get_module()
    res = run_bass_kernel_spmd(nc, in_maps, list(range(NCORES)))
    LAST_EXEC_NS = res.exec_time_ns

    p12 = np.concatenate([r["p12"] for r in res.results], 0)   # (HW, HW)
    p21 = np.concatenate([r["p21"] for r in res.results], 0)
    i12 = np.concatenate([r["i12"] for r in res.results], 0)   # (HW, 8)
    i21 = np.concatenate([r["i21"] for r in res.results], 0)

    # exact host rescore of the top-8 candidates per row
    def rescore(cand, lines, fq, fp):
        cand = np.clip(cand.astype(np.int64), 0, HW - 1)       # (HW, 8)
        a = lines[:, 0:1]; b = lines[:, 1:2]; c = lines[:, 2:3]
        xc = pts[cand[:, :], 0]                                # (HW, 8)
        yc = pts[cand[:, :], 1]
        z = -((a * xc + b * yc + c) ** 2)                      # (HW, 8) f64
        fpc = fp.astype(np.float64).T[cand]                    # (HW, 8, C)
        sc = np.einsum("qjc,qc->qj", fpc, fq.astype(np.float64).T)
        val = np.exp(z) * sc
        return cand[np.arange(HW), np.argmax(val, 1)]

    idx12 = rescore(i12, L12, f1n, f2n)
    idx21 = rescore(i21, L21, f2n, f1n)

    # reference-exact normalized grid
    xs, ys = np.meshgrid(np.arange(W), np.arange(H))
    pts32 = np.stack([xs, ys], -1).astype(np.float32) + np.float32(0.5)
    grid = (pts32 / np.array([W, H], np.float32) * np.float32(2.0)
            - np.float32(1.0)).reshape(-1, 2)

    m1_2 = grid[idx12].reshape(1, H, W, 2)
    m2_1 = grid[idx21].reshape(1, H, W, 2)
    dwcs1_2 = p12.reshape(1, H, W, H, W)
    dwcs2_1 = p21.reshape(1, H, W, H, W)
    return m1_2, m2_1, dwcs1_2, dwcs2_1


# revision 16
# speedup vs baseline: 1.0024x; 1.0024x over previous
"""Trainium2 Bass kernel for nn_MatchingMetric (epipolar matching metric).

Strategy: shard the query-pixel axis (first H*W) across 8 NeuronCores; each
core computes 288 query rows of both (HW, HW) softmax tensors plus the top-8
candidate argmax indices per row.  Per chunk of <=128 query rows:
    z[q,p]  = -d(q,p)^2                      (K=60 quad-split bf16 matmul)
    S[q,p]  = <f_q, g_p>                     (fp16 matmul, fp32 accumulate)
    dwc     = exp(z) * S                     (ACT exp + DVE multiply)
    probs   = exp(dwc) / sum_p exp(dwc)      (ACT with fused row-sum,
                                              GPSIMD normalize_recip)
    top-8 of dwc (values+indices)            (DVE max8 / max_index)
The tiny camera math, feature normalization, exact rescoring of the 8
argmax candidates per row, and the index->grid lookup run on the host.
"""
import sys

sys.path.insert(0, "/opt/trn_rl_repo")

import numpy as np
import ml_dtypes
from contextlib import ExitStack

import concourse.bacc as bacc
import concourse.tile as tile
import concourse.mybir as mybir
from concourse.bass_utils import run_bass_kernel_spmd

B, C, H, W = 1, 256, 48, 48
HW = H * W
EPS = 1e-6
NCORES = 8
QPC = HW // NCORES            # 288 query rows per core
NSLC = [(0, 512), (512, 512), (1024, 512), (1536, 512), (2048, 256)]
ZK = 60                       # K of the z matmul (10 split pairs x 6 monomials)

LAST_EXEC_NS = None
_CACHED = {}


# --------------------------------------------------------------------------
# host-side math
# --------------------------------------------------------------------------
def _bf16(x):
    return np.asarray(x, ml_dtypes.bfloat16)


def _epiline_geometry(K1, K2, c2w1, c2w2):
    """Folded-normalized epiline coefficients per query pixel (float64)."""
    def proj(K, c2w):
        R = c2w[:3, :3]
        t = c2w[:3, 3:4]
        w2c = np.concatenate([R.T, -R.T @ t], axis=1)
        return K @ w2c

    def fundamental(P1, P2):
        rows = [np.array([1, 2]), np.array([2, 0]), np.array([0, 1])]
        X = [P1[r, :] for r in rows]
        Y = [P2[r, :] for r in rows]
        mats = np.stack(
            [np.concatenate([X[j], Y[i]], 0) for i in range(3) for j in range(3)], 0
        )
        return np.linalg.det(mats).reshape(3, 3)

    P1 = proj(K1.astype(np.float64), c2w1.astype(np.float64))
    P2 = proj(K2.astype(np.float64), c2w2.astype(np.float64))
    F = fundamental(P1, P2)

    xs, ys = np.meshgrid(np.arange(W), np.arange(H))
    pts = np.stack([xs, ys], -1).reshape(-1, 2).astype(np.float64) + 0.5
    ph = np.concatenate([pts, np.ones((HW, 1))], 1)

    def lines_of(Fm):
        ln = ph @ Fm.T
        ln = ln / np.sqrt(ln[:, 0] ** 2 + ln[:, 1] ** 2)[:, None]
        ln = ln / np.sqrt(ln[:, 0] ** 2 + ln[:, 1] ** 2 + 1e-9)[:, None]
        return ln

    L21 = lines_of(F)       # lines indexed by q for direction 2->1
    L12 = lines_of(F.T)     # direction 1->2
    return L12, L21, pts


def _quad_coeffs(lines):
    a, b, c = lines[:, 0], lines[:, 1], lines[:, 2]
    x0, y0 = W / 2.0, H / 2.0
    ct = c + a * x0 + b * y0
    return np.stack([-a * a, -b * b, -ct * ct, -2 * a * b, -2 * a * ct, -2 * b * ct], 0)


def _monomials(pts):
    x0, y0 = W / 2.0, H / 2.0
    xt = pts[:, 0] - x0
    yt = pts[:, 1] - y0
    return np.stack([xt * xt, yt * yt, np.ones(HW), xt * yt, xt, yt], 0)


_SPLIT_PAIRS = [(i, j) for i in range(4) for j in range(4) if i + j < 4]  # 10 pairs


def _quad_split_stack(x64, pick):
    """4-level bf16 split of x64 (6, N); stack the `pick` level per pair -> (60, N)."""
    levels = []
    r = np.array(x64, np.float64)
    for _ in range(4):
        h = _bf16(r).astype(np.float64)
        levels.append(_bf16(r))
        r = r - h
    return np.concatenate([levels[p[pick]] for p in _SPLIT_PAIRS], 0)


def _normalized_features(features1, features2):
    f1 = np.asarray(features1, np.float32)[0].reshape(C, HW) + np.float32(EPS)
    f2 = np.asarray(features2, np.float32)[0].reshape(C, HW) + np.float32(EPS)
    n1 = np.sqrt((f1 ** 2).sum(0, dtype=np.float32))
    n2 = np.sqrt((f2 ** 2).sum(0, dtype=np.float32))
    return f1 / n1[None, :], f2 / n2[None, :]


# --------------------------------------------------------------------------
# device program (SPMD, one Bacc module shared by all 8 cores)
# --------------------------------------------------------------------------
def _build_module():
    nc = bacc.Bacc("TRN2", num_devices=NCORES)
    f16 = mybir.dt.float16
    bf16 = mybir.dt.bfloat16
    f32 = mybir.dt.float32
    u32 = mybir.dt.uint32

    ins = {}
    for nm, shape, dt in [
        ("h1", [C, HW], f16), ("h2", [C, HW], f16),
        ("h1q", [C, QPC], f16), ("h2q", [C, QPC], f16),
        ("uz", [ZK, HW], bf16),
        ("wq12", [ZK, QPC], bf16), ("wq21", [ZK, QPC], bf16),
    ]:
        ins[nm] = nc.declare_dram_parameter(nm, shape, dt, isOutput=False)

    outs = {}
    for nm, shape, dt in [
        ("p12", [QPC, HW], f32), ("p21", [QPC, HW], f32),
        ("i12", [QPC, 8], u32), ("i21", [QPC, 8], u32),
    ]:
        outs[nm] = nc.declare_dram_parameter(nm, shape, dt, isOutput=True)

    # chunk = list of (direction, q-start, partition-base, rows)
    CHUNKS = [
        [(0, 0, 0, 128)],
        [(0, 128, 0, 128)],
        [(1, 0, 0, 128)],
        [(1, 128, 0, 128)],
        [(0, 256, 0, 32), (1, 256, 32, 32)],   # merged tails
    ]
    DIR = [("wq12", "h1q", "h2", "p12", "i12"), ("wq21", "h2q", "h1", "p21", "i21")]

    with tile.TileContext(nc) as tc, ExitStack() as ctx:
        cpool = ctx.enter_context(tc.tile_pool(name="consts", bufs=1))
        wpool = ctx.enter_context(tc.tile_pool(name="work", bufs=3))
        spool = ctx.enter_context(tc.tile_pool(name="small", bufs=4))
        psz = ctx.enter_context(tc.tile_pool(name="psz", bufs=1, space="PSUM"))
        pss = ctx.enter_context(tc.tile_pool(name="pss", bufs=3, space="PSUM"))

        # loads in first-use order: chunk 0 (dir 1->2) needs wq12, uz, h1q, h2
        sb = {}

        def load_wq(nm):
            t = cpool.tile([ZK, QPC], bf16, name=nm)
            nc.sync.dma_start(t[:], ins[nm][:, :])
            sb[nm] = t

        def load_feat(nm, width):
            for k in range(2):
                t = cpool.tile([128, width], f16, name=f"{nm}_{k}")
                nc.sync.dma_start(t[:], ins[nm][k * 128:(k + 1) * 128, :])
                sb[f"{nm}_{k}"] = t

        load_wq("wq12")
        uz = cpool.tile([ZK, HW], bf16, name="uz")
        nc.sync.dma_start(uz[:], ins["uz"][:, :])
        load_feat("h1q", QPC)
        # h2 is on chunk 0's critical path: land its first half early
        for k in range(2):
            t = cpool.tile([128, HW], f16, name=f"h2_{k}")
            nc.sync.dma_start(t[:, :1024], ins["h2"][k * 128:(k + 1) * 128, :1024])
            nc.sync.dma_start(t[:, 1024:], ins["h2"][k * 128:(k + 1) * 128, 1024:])
            sb[f"h2_{k}"] = t
        load_wq("wq21")
        load_feat("h2q", QPC)
        load_feat("h1", HW)

        for ci, parts in enumerate(CHUNKS):
            sfx = f"c{ci}"
            P = sum(pp[3] for pp in parts)       # total partitions used

            # --- z matmul -> PSUM ---
            zp = psz.tile([128, HW], f32, tag="z", name=f"z_{sfx}")
            for d, qs, pb, rows in parts:
                wqd = sb[DIR[d][0]]
                for ns, nw in NSLC:
                    nc.tensor.matmul(
                        zp[pb:pb + rows, ns:ns + nw],
                        wqd[:, qs:qs + rows],
                        uz[:, ns:ns + nw],
                        start=True, stop=True,
                    )
            # --- e = exp(z) (chunk 0: halves, to start the pipeline sooner) ---
            e = wpool.tile([128, HW], f32, tag="e", name=f"e_{sfx}")
            if ci == 0:
                nc.scalar.activation(e[:P, :1024], zp[:P, :1024],
                                     mybir.ActivationFunctionType.Exp)
                nc.scalar.activation(e[:P, 1024:], zp[:P, 1024:],
                                     mybir.ActivationFunctionType.Exp)
            else:
                nc.scalar.activation(e[:P, :], zp[:P, :],
                                     mybir.ActivationFunctionType.Exp)

            # --- S matmul (fp16) + dwc = e * S per n-slice ---
            dwc = wpool.tile([128, HW], f32, tag="dwc", name=f"dwc_{sfx}")
            for ns, nw in NSLC:
                sp = pss.tile([128, 512], f32, tag="s", name=f"s_{sfx}_{ns}")
                for d, qs, pb, rows in parts:
                    _, lhs, rhs = DIR[d][0], DIR[d][1], DIR[d][2]
                    for k in range(2):
                        nc.tensor.matmul(
                            sp[pb:pb + rows, :nw],
                            sb[f"{lhs}_{k}"][:, qs:qs + rows],
                            sb[f"{rhs}_{k}"][:, ns:ns + nw],
                            start=(k == 0), stop=(k == 1),
                        )
                nc.vector.tensor_tensor(
                    dwc[:P, ns:ns + nw], e[:P, ns:ns + nw], sp[:P, :nw],
                    op=mybir.AluOpType.mult,
                )

            mx = spool.tile([128, 8], f32, tag="mx", name=f"mx_{sfx}")
            mi = spool.tile([128, 8], u32, tag="mi", name=f"mi_{sfx}")
            last = ci == len(CHUNKS) - 1
            if not last:
                # top-8 of dwc, overlapping the exp/normalize chain
                nc.vector.max_with_indices(mx[:P, :], mi[:P, :], dwc[:P, :])

            # --- u = exp(dwc), s = row sum (fused) ---
            u = wpool.tile([128, HW], f32, tag="u", name=f"u_{sfx}")
            s = spool.tile([128, 1], f32, tag="sum", name=f"sum_{sfx}")
            nc.scalar.activation(
                u[:P, :], dwc[:P, :], mybir.ActivationFunctionType.Exp,
                accum_out=s[:P, :],
            )

            # --- probs = u / s (gpsimd) ---
            p = wpool.tile([128, HW], f32, tag="p", name=f"p_{sfx}")
            nc.gpsimd.normalize_recip(p[:P, :], u[:P, :], s[:P, :])
            if last:
                # emitted after the softmax chain: the scans hide under the
                # final probs DMA instead of extending the tail
                nc.vector.max_with_indices(mx[:P, :], mi[:P, :], dwc[:P, :])

            for d, qs, pb, rows in parts:
                nc.sync.dma_start(outs[DIR[d][3]][qs:qs + rows, :], p[pb:pb + rows, :])
                nc.sync.dma_start(outs[DIR[d][4]][qs:qs + rows, :], mi[pb:pb + rows, :])

    nc.finalize()
    return nc


def _get_module():
    if "nc" not in _CACHED:
        _CACHED["nc"] = _build_module()
    return _CACHED["nc"]


# --------------------------------------------------------------------------
# entry point
# --------------------------------------------------------------------------
def kernel(features1, features2, K1, K2, c2w1, c2w2):
    global LAST_EXEC_NS
    K1 = np.asarray(K1, np.float32)[0]
    K2 = np.asarray(K2, np.float32)[0]
    c2w1 = np.asarray(c2w1, np.float32)[0]
    c2w2 = np.asarray(c2w2, np.float32)[0]

    L12, L21, pts = _epiline_geometry(K1, K2, c2w1, c2w2)
    w12 = _quad_coeffs(L12)
    w21 = _quad_coeffs(L21)
    umon = _monomials(pts)
    wz12 = _quad_split_stack(w12, 0)
    wz21 = _quad_split_stack(w21, 0)
    uzs = _quad_split_stack(umon, 1)

    f1n, f2n = _normalized_features(features1, features2)
    h1 = f1n.astype(np.float16)
    h2 = f2n.astype(np.float16)

    in_maps = []
    for cid in range(NCORES):
        sl = slice(cid * QPC, (cid + 1) * QPC)
        in_maps.append({
            "h1": h1, "h2": h2,
            "h1q": np.ascontiguousarray(h1[:, sl]),
            "h2q": np.ascontiguousarray(h2[:, sl]),
            "uz": uzs,
            "wq12": np.ascontiguousarray(wz12[:, sl]),
            "wq21": np.ascontiguousarray(wz21[:, sl]),
        })

    nc = _get_module()
    res = run_bass_kernel_spmd(nc, in_maps, list(range(NCORES)))
    LAST_EXEC_NS = res.exec_time_ns

    p12 = np.concatenate([r["p12"] for r in res.results], 0)   # (HW, HW)
    p21 = np.concatenate([r["p21"] for r in res.results], 0)
    i12 = np.concatenate([r["i12"] for r in res.results], 0)   # (HW, 8)
    i21 = np.concatenate([r["i21"] for r in res.results], 0)

    # exact host rescore of the top-8 candidates per row
    def rescore(cand, lines, fq, fp):
        cand = np.clip(cand.astype(np.int64), 0, HW - 1)       # (HW, 8)
        a = lines[:, 0:1]; b = lines[:, 1:2]; c = lines[:, 2:3]
        xc = pts[cand[:, :], 0]                                # (HW, 8)
        yc = pts[cand[:, :], 1]
        z = -((a * xc + b * yc + c) ** 2)                      # (HW, 8) f64
        fpc = fp.astype(np.float64).T[cand]                    # (HW, 8, C)
        sc = np.einsum("qjc,qc->qj", fpc, fq.astype(np.float64).T)
        val = np.exp(z) * sc
        return cand[np.arange(HW), np.argmax(val, 1)]

    idx12 = rescore(i12, L12, f1n, f2n)
    idx21 = rescore(i21, L21, f2n, f1n)

    # reference-exact normalized grid
    xs, ys = np.meshgrid(np.arange(W), np.arange(H))
    pts32 = np.stack([xs, ys], -1).astype(np.float32) + np.float32(0.5)
    grid = (pts32 / np.array([W, H], np.float32) * np.float32(2.0)
            - np.float32(1.0)).reshape(-1, 2)

    m1_2 = grid[idx12].reshape(1, H, W, 2)
    m2_1 = grid[idx21].reshape(1, H, W, 2)
    dwcs1_2 = p12.reshape(1, H, W, H, W)
    dwcs2_1 = p21.reshape(1, H, W, H, W)
    return m1_2, m2_1, dwcs1_2, dwcs2_1
